# revision 5
# baseline (speedup 1.0000x reference)
"""LIF (leaky integrate-and-fire) forward recurrence on 8 Trainium2 NeuronCores.

Input  x: (T=16, B=128, N=16384) float32, time-major.
    m[t] = tau * v[t-1] + x[t]
    y[t] = (m[t] >= v_th)            spike, as 0.0/1.0
    v[t] = m[t] * (1 - y[t])         hard reset

Sharding: N split 8 ways (2048 neurons per core); the recurrence is
per-neuron independent so cores never communicate.  Host re-lays each
shard as (B, T, N) so DMA chunks read long contiguous runs per SBUF
partition row.

Dataflow per step (all engines in parallel, all arithmetic f32-exact):

  PE     m_ps = I.T @ x[t] + (tau*I).T @ v[t-1]   (PSUM; identity and
         0.5-identity fp32 matmuls verified bit-exact on HW)
  ACT    sig = Sign(m_ps - 1) -> int8 {-1,0,+1}   <- doubles as OUTPUT
         (host maps sig>=0 -> spike) and as the reset predicate
  DVE    v[t] = (sig < 0) * m_ps                  (sig SBUF + m PSUM:
         one read each; no PSUM->SBUF copy needed anywhere)
  SYNC   input x DMA chunks (HWDGE)
  ACT(HWDGE ring) output sig chunks

sig < 0  <=>  m < 1, so v = m*(m<1) exactly; multiply by {0.0,1.0} and
the Sign comparison are exact, so the result is bit-identical to the
f32 reference.
"""

import numpy as np

import concourse.bass as bass
import concourse.mybir as mybir
from concourse.bass_utils import run_bass_kernel_spmd
from concourse.mybir import AluOpType
from concourse.tile import TileContext

T, B, N = 16, 128, 16384
NCORES = 8
NSH = N // NCORES  # 2048 neurons per core
TAU = 0.5
V_TH = 1.0

BANK = 512  # fp32 columns per PSUM bank == matmul max moving free dim
NBANK = NSH // BANK  # 4
CW = 1024  # column-block width for the ACT/DVE ops (2 banks)

IN_CHUNKS = [1, 1, 2, 4, 4, 4]
OUT_CHUNKS = [4, 4, 4, 4]

_cached_nc = None


def _split_multiwaits(nc):
    """Walrus codegen supports only ONE sync-wait per instruction; Tile
    sometimes attaches more.  Move extras onto same-engine NoOps inserted
    just before (sequencer runs in program order, so semantics hold)."""
    multi_ok = (mybir.InstEventSemaphore, mybir.InstNoOp)
    for f in nc.m.functions:
        for b in f.blocks:
            new_insts = []
            for inst in b.instructions:
                si = inst.sync_info
                if (
                    not isinstance(inst, multi_ok)
                    and si is not None
                    and len(si.on_wait) > 1
                ):
                    waits = list(si.on_wait)
                    for j, w in enumerate(waits[:-1]):
                        new_insts.append(
                            mybir.InstNoOp(
                                name=f"{inst.name}_presync{j}",
                                engine=inst.engine,
                                sync_info=mybir.SyncInfo(on_wait=[w], on_update=[]),
                            )
                        )
                    inst.sync_info = mybir.SyncInfo(
                        on_wait=[waits[-1]], on_update=list(si.on_update)
                    )
                new_insts.append(inst)
            b.instructions = new_insts


def _build():
    nc = bass.Bass(trn_type="TRN2")
    x = nc.dram_tensor("x", [B, T, NSH], mybir.dt.float32, kind="ExternalInput")
    ident = nc.dram_tensor(
        "ident", [B, 2, B], mybir.dt.float32, kind="ExternalInput"
    )  # [:,0,:] = I, [:,1,:] = tau*I
    y = nc.dram_tensor("y", [B, T, NSH], mybir.dt.int8, kind="ExternalOutput")

    with TileContext(nc) as tc:
        with (
            tc.tile_pool(name="const", bufs=1) as const_pool,
            tc.tile_pool(name="state", bufs=1) as state_pool,
            tc.tile_pool(name="xin", bufs=2) as xin_pool,
            tc.tile_pool(name="yout", bufs=2) as yout_pool,
            tc.tile_pool(name="psum", bufs=1, space="PSUM") as psum_pool,
        ):
            idt = const_pool.tile([B, 2, B], mybir.dt.float32)
            nc.sync.dma_start(out=idt[:], in_=ident[:])

            v_t = [
                state_pool.tile([B, NSH], mybir.dt.float32, name=f"v{p}")
                for p in range(2)
            ]
            m_ps = [
                psum_pool.tile([B, NSH], mybir.dt.float32, name=f"mps{p}")
                for p in range(2)
            ]

            xt_tiles = {}
            t0 = 0
            for ci, w in enumerate(IN_CHUNKS):
                xt = xin_pool.tile(
                    [B, 4, NSH], mybir.dt.float32, tag="xt", name=f"xt{ci}"
                )
                nc.sync.dma_start(out=xt[:, :w, :], in_=x[:, t0 : t0 + w, :])
                for k in range(w):
                    xt_tiles[t0 + k] = xt[:, k, :]
                t0 += w

            out_t0 = 0
            oc = 0
            yt = None
            for t in range(T):
                p = t % 2
                if yt is None:
                    yt = yout_pool.tile(
                        [B, 4, NSH], mybir.dt.int8, tag="yt", name=f"yt{oc}"
                    )
                xk = xt_tiles[t]
                # m = I.T @ x[t] (+ (tau I).T @ v[t-1]) into PSUM, per bank
                for k in range(NBANK):
                    sl = slice(k * BANK, (k + 1) * BANK)
                    nc.tensor.matmul(
                        m_ps[p][:, sl],
                        idt[:, 0, :],
                        xk[:, sl],
                        start=True,
                        stop=(t == 0),
                    )
                    if t > 0:
                        nc.tensor.matmul(
                            m_ps[p][:, sl],
                            idt[:, 1, :],
                            v_t[1 - p][:, sl],
                            start=False,
                            stop=True,
                        )
                # per column-block: sig on ACT, then v on DVE
                for c0 in range(0, NSH, CW):
                    cs = slice(c0, c0 + CW)
                    # sig = Sign(1 - m) in {+1, 0, -1} as int8; this is the
                    # output chunk (host: spike = sig <= 0) AND the reset
                    # predicate (sig > 0  <=>  m < 1)
                    nc.scalar.activation(
                        yt[:, t - out_t0, cs],
                        m_ps[p][:, cs],
                        mybir.ActivationFunctionType.Sign,
                        bias=V_TH,
                        scale=-1.0,
                    )
                    # v = (sig > 0) * m    (exact hard reset)
                    nc.vector.scalar_tensor_tensor(
                        v_t[p][:, cs],
                        yt[:, t - out_t0, cs],
                        0.0,
                        m_ps[p][:, cs],
                        AluOpType.is_gt,
                        AluOpType.mult,
                    )
                if t - out_t0 + 1 == OUT_CHUNKS[oc]:
                    w = OUT_CHUNKS[oc]
                    nc.scalar.dma_start(
                        out=y[:, out_t0 : out_t0 + w, :], in_=yt[:, :w, :]
                    )
                    out_t0 += w
                    oc += 1
                    yt = None
    _split_multiwaits(nc)
    return nc


_IDENT = None


def kernel(x: np.ndarray) -> np.ndarray:
    global _cached_nc, _IDENT
    if _cached_nc is None:
        _cached_nc = _build()
    nc = _cached_nc
    if _IDENT is None:
        _IDENT = np.zeros((B, 2, B), dtype=np.float32)
        _IDENT[:, 0, :] = np.eye(B, dtype=np.float32)
        _IDENT[:, 1, :] = np.eye(B, dtype=np.float32) * np.float32(TAU)

    x = np.ascontiguousarray(x, dtype=np.float32)
    assert x.shape == (T, B, N)
    # (T, B, N) -> per-core (B, T, NSH) shards, timestep-contiguous rows
    xbt = np.ascontiguousarray(x.transpose(1, 0, 2))
    in_maps = [
        {
            "x": np.ascontiguousarray(xbt[:, :, k * NSH : (k + 1) * NSH]),
            "ident": _IDENT,
        }
        for k in range(NCORES)
    ]
    res = run_bass_kernel_spmd(nc, in_maps, core_ids=list(range(NCORES)))
    global _last_exec_ns
    if res.exec_time_ns is not None:
        _last_exec_ns = res.exec_time_ns
    # per-core int8 sign (B, T, NSH): sig = Sign(1-m), spike <=> sig <= 0
    out = np.concatenate([r["y"] for r in res.results], axis=2)
    return (
        np.ascontiguousarray(out.transpose(1, 0, 2)) <= 0
    ).astype(np.float32)


_last_exec_ns = None


# revision 6
# speedup vs baseline: 1.6667x; 1.6667x over previous
"""LIF (leaky integrate-and-fire) forward recurrence on 8 Trainium2 NeuronCores.

Input  x: (T=16, B=128, N=16384) float32, time-major.
    m[t] = tau * v[t-1] + x[t]
    y[t] = (m[t] >= v_th)            spike, as 0.0/1.0
    v[t] = m[t] * (1 - y[t])         hard reset

Sharding: N split 8 ways (2048 per core); the recurrence is per-neuron
independent so the cores never communicate.  The host re-lays each shard
as (B, T, N) so a multi-timestep DMA chunk reads/writes long contiguous
runs per SBUF partition row.

Per core per timestep the work is a [128 x 2048] f32 tile:
    m   = (v * tau) + x[t]       scalar_tensor_tensor on DVE
    sig = Sign(1 - m)            ScalarE -> int8 {+1,0,-1}; the OUTPUT
                                 (host: spike = sig <= 0) - one ACT op
                                 per step instead of two
    v'  = (m < 1) * m            scalar_tensor_tensor on DVE

The DVE pair (m, v') is the critical path; both read only m/v/x so the
chain never waits on the Scalar engine.  All ops are exact in f32, so
the result is bit-identical to the f32 reference.
"""

import numpy as np

import concourse.bass as bass
import concourse.mybir as mybir
from concourse.bass_utils import run_bass_kernel_spmd
from concourse.mybir import AluOpType
from concourse.tile import TileContext

T, B, N = 16, 128, 16384
NCORES = 8
NSH = N // NCORES  # 2048 neurons per core
TAU = 0.5
V_TH = 1.0

IN_CHUNKS = [1, 1, 2, 4, 4, 4]
OUT_CHUNKS = [4, 4, 4, 2, 1, 1]

_cached_nc = None


def _split_multiwaits(nc):
    """Walrus codegen in this toolchain supports only ONE sync-wait per
    instruction (single wait slot in the EVENTS field); Tile sometimes
    attaches two or more.  Move the extra waits onto same-engine NoOps
    inserted right before - the sequencer executes in program order, so
    semantics are unchanged."""
    multi_ok = (mybir.InstEventSemaphore, mybir.InstNoOp)
    for f in nc.m.functions:
        for b in f.blocks:
            new_insts = []
            for inst in b.instructions:
                si = inst.sync_info
                if (
                    not isinstance(inst, multi_ok)
                    and si is not None
                    and len(si.on_wait) > 1
                ):
                    waits = list(si.on_wait)
                    for j, w in enumerate(waits[:-1]):
                        new_insts.append(
                            mybir.InstNoOp(
                                name=f"{inst.name}_presync{j}",
                                engine=inst.engine,
                                sync_info=mybir.SyncInfo(on_wait=[w], on_update=[]),
                            )
                        )
                    inst.sync_info = mybir.SyncInfo(
                        on_wait=[waits[-1]], on_update=list(si.on_update)
                    )
                new_insts.append(inst)
            b.instructions = new_insts


def _build():
    nc = bass.Bass(trn_type="TRN2")
    x = nc.dram_tensor("x", [B, T, NSH], mybir.dt.float32, kind="ExternalInput")
    y = nc.dram_tensor("y", [B, T, NSH], mybir.dt.int8, kind="ExternalOutput")

    with TileContext(nc) as tc:
        with (
            tc.tile_pool(name="state", bufs=1) as state_pool,
            tc.tile_pool(name="xin", bufs=2) as xin_pool,
            tc.tile_pool(name="yout", bufs=2) as yout_pool,
            tc.tile_pool(name="work", bufs=3) as work_pool,
        ):
            v = state_pool.tile([B, NSH], mybir.dt.float32)

            xt_tiles = {}
            t0 = 0
            for ci, w in enumerate(IN_CHUNKS):
                xt = xin_pool.tile(
                    [B, 4, NSH], mybir.dt.float32, tag="xt", name=f"xt{ci}"
                )
                # input loads on the gpsimd SWDGE path so they don't queue
                # behind Sync's preamble
                nc.gpsimd.dma_start(out=xt[:, :w, :], in_=x[:, t0 : t0 + w, :])
                for k in range(w):
                    xt_tiles[t0 + k] = xt[:, k, :]
                t0 += w

            out_t0 = 0
            oc = 0
            yt = None
            for t in range(T):
                if yt is None:
                    yt = yout_pool.tile(
                        [B, 4, NSH], mybir.dt.int8, tag="yt", name=f"yt{oc}"
                    )
                xk = xt_tiles[t]
                if t == 0:
                    m = xk  # v==0 -> m = x[0]
                else:
                    mt = work_pool.tile(
                        [B, NSH], mybir.dt.float32, tag="m", name="mt"
                    )
                    # m = v * tau + x[t]
                    nc.vector.scalar_tensor_tensor(
                        mt[:], v[:], TAU, xk, AluOpType.mult, AluOpType.add
                    )
                    m = mt[:]
                # sig = Sign(1 - m) -> int8; host: spike = (sig <= 0)
                nc.scalar.activation(
                    yt[:, t - out_t0, :], m,
                    mybir.ActivationFunctionType.Sign,
                    bias=V_TH, scale=-1.0,
                )
                # v = (m < v_th) * m   (hard reset; off the ACT path)
                nc.vector.scalar_tensor_tensor(
                    v[:], m, V_TH, m, AluOpType.is_lt, AluOpType.mult
                )
                if t - out_t0 + 1 == OUT_CHUNKS[oc]:
                    w = OUT_CHUNKS[oc]
                    nc.sync.dma_start(
                        out=y[:, out_t0 : out_t0 + w, :], in_=yt[:, :w, :]
                    )
                    out_t0 += w
                    oc += 1
                    yt = None
    _split_multiwaits(nc)
    return nc


def kernel(x: np.ndarray) -> np.ndarray:
    global _cached_nc
    if _cached_nc is None:
        _cached_nc = _build()
    nc = _cached_nc

    x = np.ascontiguousarray(x, dtype=np.float32)
    assert x.shape == (T, B, N)
    # (T, B, N) -> per-core (B, T, NSH) shards, timestep-contiguous rows
    xbt = np.ascontiguousarray(x.transpose(1, 0, 2))
    in_maps = [
        {"x": np.ascontiguousarray(xbt[:, :, k * NSH : (k + 1) * NSH])}
        for k in range(NCORES)
    ]
    res = run_bass_kernel_spmd(nc, in_maps, core_ids=list(range(NCORES)))
    global _last_exec_ns
    if res.exec_time_ns is not None:
        _last_exec_ns = res.exec_time_ns
    # per-core int8 sign (B, T, NSH): sig = Sign(1-m), spike <=> sig <= 0
    out = np.concatenate([r["y"] for r in res.results], axis=2)
    return (
        np.ascontiguousarray(out.transpose(1, 0, 2)) <= 0
    ).astype(np.float32)


_last_exec_ns = None


# revision 8
# speedup vs baseline: 1.7568x; 1.0541x over previous
"""LIF (leaky integrate-and-fire) forward recurrence on 8 Trainium2 NeuronCores.

Input  x: (T=16, B=128, N=16384) float32, time-major.
    m[t] = tau * v[t-1] + x[t]
    y[t] = (m[t] >= v_th)            spike, as 0.0/1.0
    v[t] = m[t] * (1 - y[t])         hard reset

Sharding: N split 8 ways (2048 per core); the recurrence is per-neuron
independent so the cores never communicate.  The host re-lays each shard
as (B, T, N) so a multi-timestep DMA chunk reads/writes long contiguous
runs per SBUF partition row.

Per core per timestep the work is a [128 x 2048] f32 tile:
    m   = (v * tau) + x[t]       scalar_tensor_tensor on DVE
    sig = Sign(1 - m)            ScalarE -> int8 {+1,0,-1}; the OUTPUT
                                 (host: spike = sig <= 0) - one ACT op
                                 per step instead of two
    v'  = (m < 1) * m            scalar_tensor_tensor on DVE

The DVE pair (m, v') is the critical path; both read only m/v/x so the
chain never waits on the Scalar engine.  All ops are exact in f32, so
the result is bit-identical to the f32 reference.
"""

import numpy as np

import concourse.bass as bass
import concourse.mybir as mybir
from concourse.bass_utils import run_bass_kernel_spmd
from concourse.mybir import AluOpType
from concourse.tile import TileContext

T, B, N = 16, 128, 16384
NCORES = 8
NSH = N // NCORES  # 2048 neurons per core
TAU = 0.5
V_TH = 1.0

IN_CHUNKS = [1, 1, 2, 2, 2, 4, 4]
OUT_CHUNKS = [4, 4, 4, 2, 1, 1]

_cached_nc = None


def _split_multiwaits(nc):
    """Walrus codegen in this toolchain supports only ONE sync-wait per
    instruction (single wait slot in the EVENTS field); Tile sometimes
    attaches two or more.  Move the extra waits onto same-engine NoOps
    inserted right before - the sequencer executes in program order, so
    semantics are unchanged."""
    multi_ok = (mybir.InstEventSemaphore, mybir.InstNoOp)
    for f in nc.m.functions:
        for b in f.blocks:
            new_insts = []
            for inst in b.instructions:
                si = inst.sync_info
                if (
                    not isinstance(inst, multi_ok)
                    and si is not None
                    and len(si.on_wait) > 1
                ):
                    waits = list(si.on_wait)
                    for j, w in enumerate(waits[:-1]):
                        new_insts.append(
                            mybir.InstNoOp(
                                name=f"{inst.name}_presync{j}",
                                engine=inst.engine,
                                sync_info=mybir.SyncInfo(on_wait=[w], on_update=[]),
                            )
                        )
                    inst.sync_info = mybir.SyncInfo(
                        on_wait=[waits[-1]], on_update=list(si.on_update)
                    )
                new_insts.append(inst)
            b.instructions = new_insts


def _build():
    nc = bass.Bass(trn_type="TRN2")
    x = nc.dram_tensor("x", [B, T, NSH], mybir.dt.float32, kind="ExternalInput")
    y = nc.dram_tensor("y", [B, T, NSH], mybir.dt.int8, kind="ExternalOutput")

    with TileContext(nc) as tc:
        with (
            tc.tile_pool(name="state", bufs=1) as state_pool,
            tc.tile_pool(name="xin", bufs=2) as xin_pool,
            tc.tile_pool(name="yout", bufs=2) as yout_pool,
            tc.tile_pool(name="work", bufs=3) as work_pool,
        ):
            v = state_pool.tile([B, NSH], mybir.dt.float32)

            xt_tiles = {}
            t0 = 0
            for ci, w in enumerate(IN_CHUNKS):
                xt = xin_pool.tile(
                    [B, 4, NSH], mybir.dt.float32, tag="xt", name=f"xt{ci}"
                )
                # input loads on the Scalar HWDGE ring: ~0.6us first-byte
                # latency and none of the gpsimd SWDGE preamble/drain cost
                nc.scalar.dma_start(out=xt[:, :w, :], in_=x[:, t0 : t0 + w, :])
                for k in range(w):
                    xt_tiles[t0 + k] = xt[:, k, :]
                t0 += w

            out_t0 = 0
            oc = 0
            yt = None
            for t in range(T):
                if yt is None:
                    yt = yout_pool.tile(
                        [B, 4, NSH], mybir.dt.int8, tag="yt", name=f"yt{oc}"
                    )
                xk = xt_tiles[t]
                if t == 0:
                    m = xk  # v==0 -> m = x[0]
                else:
                    mt = work_pool.tile(
                        [B, NSH], mybir.dt.float32, tag="m", name="mt"
                    )
                    # m = v * tau + x[t]
                    nc.vector.scalar_tensor_tensor(
                        mt[:], v[:], TAU, xk, AluOpType.mult, AluOpType.add
                    )
                    m = mt[:]
                # sig = Sign(1 - m) -> int8; host: spike = (sig <= 0)
                nc.scalar.activation(
                    yt[:, t - out_t0, :], m,
                    mybir.ActivationFunctionType.Sign,
                    bias=V_TH, scale=-1.0,
                )
                # v = (m < v_th) * m   (hard reset; off the ACT path)
                nc.vector.scalar_tensor_tensor(
                    v[:], m, V_TH, m, AluOpType.is_lt, AluOpType.mult
                )
                if t - out_t0 + 1 == OUT_CHUNKS[oc]:
                    w = OUT_CHUNKS[oc]
                    nc.sync.dma_start(
                        out=y[:, out_t0 : out_t0 + w, :], in_=yt[:, :w, :]
                    )
                    out_t0 += w
                    oc += 1
                    yt = None
    _split_multiwaits(nc)
    return nc


def kernel(x: np.ndarray) -> np.ndarray:
    global _cached_nc
    if _cached_nc is None:
        _cached_nc = _build()
    nc = _cached_nc

    x = np.ascontiguousarray(x, dtype=np.float32)
    assert x.shape == (T, B, N)
    # (T, B, N) -> per-core (B, T, NSH) shards, timestep-contiguous rows
    xbt = np.ascontiguousarray(x.transpose(1, 0, 2))
    in_maps = [
        {"x": np.ascontiguousarray(xbt[:, :, k * NSH : (k + 1) * NSH])}
        for k in range(NCORES)
    ]
    res = run_bass_kernel_spmd(nc, in_maps, core_ids=list(range(NCORES)))
    global _last_exec_ns
    if res.exec_time_ns is not None:
        _last_exec_ns = res.exec_time_ns
    # per-core int8 sign (B, T, NSH): sig = Sign(1-m), spike <=> sig <= 0
    out = np.concatenate([r["y"] for r in res.results], axis=2)
    return (
        np.ascontiguousarray(out.transpose(1, 0, 2)) <= 0
    ).astype(np.float32)


_last_exec_ns = None
